# revision 3
# baseline (speedup 1.0000x reference)
"""Segment-mean (CGPooling) Trainium2 kernel.

out[s, d] = mean over atoms i with segment_ids[i] == s of atom_features[i, d]

N = 2097152 atoms, D = 128 features, B = 8192 segments, 8 NeuronCores.

Strategy (memory-bound; roofline = reading 1 GiB of features):
- Atoms sharded across 8 cores (262144 atoms each), segment_ids sorted.
- Host splits f32 features into bf16 hi/lo halves (sum is exact to ~2^-16
  relative) and packs rows of [hi(128) | lo(128) | 1.0 | pad] = 258 bf16.
  Rows are pre-tiled so each core's DMA is per-partition contiguous.
- Device: for each 128-atom tile, build a one-hot (atom x 128-seg window)
  matrix on the vector engine (tensor_scalar is_equal vs an iota), then a
  single bf16 matmul accumulates [seg x (hi|lo|count)] into PSUM across a
  128-tile window. Windows cover 128 segments with stride 64 (+-32 margin
  for the deviation of sorted-uniform ids from their expected positions;
  verified on the host, with a numpy fallback if ever violated).
- Window flushes add PSUM into a core-local accumulator in SBUF (odd
  windows need a 64-partition rotation, done with an SBUF->SBUF DMA).
- Per-core partial sums/counts are written out; the host reduces across
  cores and divides (cheap: 8 x 4.5 MiB).
"""

import numpy as np
import ml_dtypes

BF16 = ml_dtypes.bfloat16

N = 2_097_152
D = 128
B = 8192
NCORES = 8
APC = N // NCORES  # atoms per core
TPC = APC // 128  # 2048 tiles per core
WT = 128  # tiles per window
WPC = TPC // WT  # 16 windows per core
NWIN = NCORES * WPC  # 128 global windows
ROW = 258  # hi(128) | lo(128) | ones(1) | pad(1)
NBLK = 9  # local accumulator blocks of 128 segs
CHUNK_T = 64  # tiles per DMA chunk (64*258*2B = 33 KB per partition)

_CACHE = {}


def _build_bass():
    from contextlib import ExitStack

    import concourse.tile as tile
    from concourse import bacc, mybir

    nc = bacc.Bacc("TRN2", target_bir_lowering=False, debug=False, num_devices=NCORES)
    f32 = mybir.dt.float32
    bf16 = mybir.dt.bfloat16

    hl = nc.dram_tensor("hl", [128, TPC * ROW], bf16, kind="ExternalInput").ap()
    rel = nc.dram_tensor("rel", [128, TPC], f32, kind="ExternalInput").ap()
    sums = nc.dram_tensor("sums", [128, NBLK * 128], f32, kind="ExternalOutput").ap()
    cnts = nc.dram_tensor("cnts", [128, NBLK], f32, kind="ExternalOutput").ap()

    with tile.TileContext(nc) as tc, ExitStack() as ctx:
        const_pool = ctx.enter_context(tc.tile_pool(name="const", bufs=1))
        chunk_pool = ctx.enter_context(tc.tile_pool(name="chunk", bufs=3))
        oh_pool = ctx.enter_context(tc.tile_pool(name="oh", bufs=4))
        psum_pool = ctx.enter_context(tc.tile_pool(name="psum", bufs=2, space="PSUM"))
        tmp_pool = ctx.enter_context(tc.tile_pool(name="tmp", bufs=2))
        acc_pool = ctx.enter_context(tc.tile_pool(name="acc", bufs=1))

        iota_t = const_pool.tile([128, 128], bf16)
        nc.gpsimd.iota(
            iota_t[:],
            [[1, 128]],
            channel_multiplier=0,
            allow_small_or_imprecise_dtypes=True,
        )
        rel_t = const_pool.tile([128, TPC], f32)
        nc.sync.dma_start(rel_t[:], rel[:, :])

        acc = acc_pool.tile([128, NBLK * 128], f32)
        acc_c = acc_pool.tile([128, NBLK], f32)
        nc.vector.memset(acc[:], 0.0)
        nc.vector.memset(acc_c[:], 0.0)

        chunk = None
        for w in range(WPC):
            psum = psum_pool.tile([128, ROW], f32)
            for j in range(WT):
                t = w * WT + j
                ci, cj = divmod(t, CHUNK_T)
                if cj == 0:
                    chunk = chunk_pool.tile([128, CHUNK_T * ROW], bf16)
                    nc.sync.dma_start(
                        chunk[:], hl[:, ci * CHUNK_T * ROW : (ci + 1) * CHUNK_T * ROW]
                    )
                oh = oh_pool.tile([128, 128], bf16)
                nc.vector.tensor_scalar(
                    oh[:],
                    iota_t[:],
                    rel_t[:, t : t + 1],
                    None,
                    op0=mybir.AluOpType.is_equal,
                )
                nc.tensor.matmul(
                    psum[:],
                    oh[:],
                    chunk[:, cj * ROW : (cj + 1) * ROW],
                    start=(j == 0),
                    stop=(j == WT - 1),
                )

            # Flush window w: psum partition p holds local seg ls = 64*w + p,
            # summed as [hi | lo | count]. acc block b = ls // 128, part = ls % 128.
            tmp = tmp_pool.tile([128, 130], f32)
            nc.any.tensor_copy(tmp[:, 0:128], psum[:, 0:128])
            nc.any.tensor_add(tmp[:, 0:128], tmp[:, 0:128], psum[:, 128:256])
            nc.any.tensor_copy(tmp[:, 128:129], psum[:, 256:257])
            if w % 2 == 0:
                m = w // 2
                nc.any.tensor_add(
                    acc[:, m * 128 : (m + 1) * 128],
                    acc[:, m * 128 : (m + 1) * 128],
                    tmp[:, 0:128],
                )
                nc.any.tensor_add(acc_c[:, m : m + 1], acc_c[:, m : m + 1], tmp[:, 128:129])
            else:
                m = (w - 1) // 2
                # ls = 128*m + 64 + p: rows [0:64) -> block m parts [64:128),
                # rows [64:128) -> block m+1 parts [0:64). Rotate partitions
                # by 64 via SBUF->SBUF DMA, then block-aligned adds.
                tmp2 = tmp_pool.tile([128, 130], f32)
                nc.sync.dma_start(tmp2[64:128, :], tmp[0:64, :])
                nc.sync.dma_start(tmp2[0:64, :], tmp[64:128, :])
                nc.any.tensor_add(
                    acc[64:128, m * 128 : (m + 1) * 128],
                    acc[64:128, m * 128 : (m + 1) * 128],
                    tmp2[64:128, 0:128],
                )
                nc.any.tensor_add(
                    acc[0:64, (m + 1) * 128 : (m + 2) * 128],
                    acc[0:64, (m + 1) * 128 : (m + 2) * 128],
                    tmp2[0:64, 0:128],
                )
                nc.any.tensor_add(
                    acc_c[64:128, m : m + 1], acc_c[64:128, m : m + 1], tmp2[64:128, 128:129]
                )
                nc.any.tensor_add(
                    acc_c[0:64, m + 1 : m + 2], acc_c[0:64, m + 1 : m + 2], tmp2[0:64, 128:129]
                )

        nc.sync.dma_start(sums[:, :], acc[:])
        nc.sync.dma_start(cnts[:, :], acc_c[:])

    nc.compile()
    return nc


def _get_nc():
    if "nc" not in _CACHE:
        _CACHE["nc"] = _build_bass()
    return _CACHE["nc"]


def _host_prep(feat, ids):
    """Returns (in_maps, ok). ok=False means window margins were violated."""
    # Window w covers global segs [64w - 32, 64w + 96); tile g belongs to
    # window g // 128. All ids of tile g must fall inside its window.
    g_base = 64 * (np.arange(N // 128, dtype=np.int64) // WT) - 32
    rel = ids.reshape(N // 128, 128) - g_base[:, None]
    if rel.min() < 0 or rel.max() > 127:
        return None, False
    rel_bf = rel.astype(np.float32)
    # (ntiles, 128) -> per-core (128, TPC)
    rel_cores = np.ascontiguousarray(
        rel_bf.reshape(NCORES, TPC, 128).transpose(0, 2, 1)
    )

    hi = feat.astype(BF16)
    lo = (feat - hi.astype(np.float32)).astype(BF16)
    hl = np.empty((N, ROW), dtype=BF16)
    hl[:, 0:128] = hi
    del hi
    hl[:, 128:256] = lo
    del lo
    hl[:, 256] = BF16(1.0)
    hl[:, 257] = BF16(0.0)
    # (N, ROW) -> per-core tiled (128, TPC*ROW): [p, t*ROW + c] = hl[128t + p, c]
    hl_cores = np.ascontiguousarray(
        hl.reshape(NCORES, TPC, 128, ROW).transpose(0, 2, 1, 3)
    ).reshape(NCORES, 128, TPC * ROW)
    del hl

    in_maps = [
        {"hl": hl_cores[c], "rel": rel_cores[c]} for c in range(NCORES)
    ]
    return in_maps, True


def _numpy_fallback(feat, ids, num_segments):
    sums = np.zeros((num_segments, D), dtype=np.float32)
    np.add.at(sums, ids, feat)
    counts = np.bincount(ids, minlength=num_segments).astype(np.float32)
    return sums / np.maximum(counts, 1.0)[:, None]


def kernel(atom_features, segment_ids, num_segments):
    feat = np.asarray(atom_features, dtype=np.float32)
    ids = np.asarray(segment_ids, dtype=np.int64)
    nseg = int(num_segments)
    assert feat.shape == (N, D) and ids.shape == (N,) and nseg == B, (
        feat.shape,
        ids.shape,
        nseg,
    )

    in_maps, ok = _host_prep(feat, ids)
    if not ok:
        return _numpy_fallback(feat, ids, nseg)

    from concourse.bass_utils import run_bass_kernel_spmd

    nc = _get_nc()
    res = run_bass_kernel_spmd(nc, in_maps, core_ids=list(range(NCORES)))

    sums = np.zeros((B, D), dtype=np.float32)
    counts = np.zeros((B,), dtype=np.float32)
    for c in range(NCORES):
        # local seg ls = 128*b + p maps to global s = 1024c - 32 + ls
        loc = (
            res.results[c]["sums"]
            .reshape(128, NBLK, 128)
            .transpose(1, 0, 2)
            .reshape(NBLK * 128, 128)
        )
        loc_c = res.results[c]["cnts"].transpose(1, 0).reshape(NBLK * 128)
        s = 1024 * c - 32 + np.arange(NBLK * 128)
        valid = (s >= 0) & (s < B)
        sums[s[valid]] += loc[valid]
        counts[s[valid]] += loc_c[valid]

    return sums / np.maximum(counts, 1.0)[:, None]


# revision 9
# speedup vs baseline: 168.9350x; 168.9350x over previous
"""Segment-mean (CGPooling) Trainium2 kernel.

out[s, d] = mean over atoms i with segment_ids[i] == s of atom_features[i, d]

N = 2097152 atoms, D = 128 features, B = 8192 segments, 8 NeuronCores.

Strategy (memory-bound; roofline = reading 1 GiB of features):
- Atoms sharded across 8 cores (262144 atoms each), segment_ids sorted.
- Host splits f32 features into bf16 hi/lo halves (sum is exact to ~2^-16
  relative) and packs rows of [hi(128) | lo(128) | 1.0 | pad] = 258 bf16.
  Rows are pre-tiled so each core's DMA is per-partition contiguous.
- Device: for each 128-atom tile, build a one-hot (atom x 128-seg window)
  matrix on the vector engine (tensor_scalar is_equal vs an iota), then a
  single bf16 matmul accumulates [seg x (hi|lo|count)] into PSUM across a
  128-tile window. Windows cover 128 segments with stride 64 (+-32 margin
  for the deviation of sorted-uniform ids from their expected positions;
  verified on the host, with a numpy fallback if ever violated).
- Window flushes add PSUM into a core-local accumulator in SBUF (odd
  windows need a 64-partition rotation, done with an SBUF->SBUF DMA).
- Per-core partial sums/counts are written out; the host reduces across
  cores and divides (cheap: 8 x 4.5 MiB).
"""

import numpy as np
import ml_dtypes

BF16 = ml_dtypes.bfloat16

N = 2_097_152
D = 128
B = 8192
NCORES = 8
APC = N // NCORES  # atoms per core
TPC = APC // 128  # 2048 tiles per core
WT = 128  # tiles per window
WPC = TPC // WT  # 16 windows per core
NWIN = NCORES * WPC  # 128 global windows
ROW = 258  # hi(128) | lo(128) | ones(1) | pad(1)
NBLK = 9  # local accumulator blocks of 128 segs
CHUNK_T = 64  # tiles per DMA chunk (64*258*2B = 33 KB per partition)

_CACHE = {}


def _build_bass():
    from contextlib import ExitStack

    import concourse.tile as tile
    from concourse import bacc, mybir

    nc = bacc.Bacc("TRN2", target_bir_lowering=False, debug=False, num_devices=NCORES)
    f32 = mybir.dt.float32
    bf16 = mybir.dt.bfloat16

    hl = nc.dram_tensor("hl", [128, TPC * ROW], bf16, kind="ExternalInput").ap()
    rel = nc.dram_tensor("rel", [128, TPC], f32, kind="ExternalInput").ap()
    sums = nc.dram_tensor("sums", [128, NBLK * 128], f32, kind="ExternalOutput").ap()
    cnts = nc.dram_tensor("cnts", [128, NBLK], f32, kind="ExternalOutput").ap()

    with tile.TileContext(nc) as tc, ExitStack() as ctx:
        const_pool = ctx.enter_context(tc.tile_pool(name="const", bufs=1))
        chunk_pool = ctx.enter_context(tc.tile_pool(name="chunk", bufs=3))
        oh_pool = ctx.enter_context(tc.tile_pool(name="oh", bufs=4))
        psum_pool = ctx.enter_context(tc.tile_pool(name="psum", bufs=2, space="PSUM"))
        tmp_pool = ctx.enter_context(tc.tile_pool(name="tmp", bufs=2))
        acc_pool = ctx.enter_context(tc.tile_pool(name="acc", bufs=1))

        iota_t = const_pool.tile([128, 128], bf16)
        nc.gpsimd.iota(
            iota_t[:],
            [[1, 128]],
            channel_multiplier=0,
            allow_small_or_imprecise_dtypes=True,
        )
        rel_t = const_pool.tile([128, TPC], f32)
        nc.sync.dma_start(rel_t[:], rel[:, :])

        acc = acc_pool.tile([128, NBLK * 128], f32)
        acc_c = acc_pool.tile([128, NBLK], f32)
        nc.vector.memset(acc[:], 0.0)
        nc.vector.memset(acc_c[:], 0.0)

        chunk = None
        for w in range(WPC):
            psum = psum_pool.tile([128, ROW], f32)
            for j in range(WT):
                t = w * WT + j
                ci, cj = divmod(t, CHUNK_T)
                if cj == 0:
                    chunk = chunk_pool.tile([128, CHUNK_T * ROW], bf16)
                    nc.sync.dma_start(
                        chunk[:], hl[:, ci * CHUNK_T * ROW : (ci + 1) * CHUNK_T * ROW]
                    )
                oh = oh_pool.tile([128, 128], bf16)
                nc.vector.tensor_scalar(
                    oh[:],
                    iota_t[:],
                    rel_t[:, t : t + 1],
                    None,
                    op0=mybir.AluOpType.is_equal,
                )
                nc.tensor.matmul(
                    psum[:],
                    oh[:],
                    chunk[:, cj * ROW : (cj + 1) * ROW],
                    start=(j == 0),
                    stop=(j == WT - 1),
                )

            # Flush window w: psum partition p holds local seg ls = 64*w + p,
            # summed as [hi | lo | count]. acc block b = ls // 128, part = ls % 128.
            tmp = tmp_pool.tile([128, 130], f32)
            nc.any.tensor_copy(tmp[:, 0:128], psum[:, 0:128])
            nc.any.tensor_add(tmp[:, 0:128], tmp[:, 0:128], psum[:, 128:256])
            nc.any.tensor_copy(tmp[:, 128:129], psum[:, 256:257])
            if w % 2 == 0:
                m = w // 2
                nc.any.tensor_add(
                    acc[:, m * 128 : (m + 1) * 128],
                    acc[:, m * 128 : (m + 1) * 128],
                    tmp[:, 0:128],
                )
                nc.any.tensor_add(acc_c[:, m : m + 1], acc_c[:, m : m + 1], tmp[:, 128:129])
            else:
                m = (w - 1) // 2
                # ls = 128*m + 64 + p: rows [0:64) -> block m parts [64:128),
                # rows [64:128) -> block m+1 parts [0:64). Rotate partitions
                # by 64 via SBUF->SBUF DMA, then block-aligned adds.
                tmp2 = tmp_pool.tile([128, 130], f32)
                nc.sync.dma_start(tmp2[64:128, :], tmp[0:64, :])
                nc.sync.dma_start(tmp2[0:64, :], tmp[64:128, :])
                nc.any.tensor_add(
                    acc[64:128, m * 128 : (m + 1) * 128],
                    acc[64:128, m * 128 : (m + 1) * 128],
                    tmp2[64:128, 0:128],
                )
                nc.any.tensor_add(
                    acc[0:64, (m + 1) * 128 : (m + 2) * 128],
                    acc[0:64, (m + 1) * 128 : (m + 2) * 128],
                    tmp2[0:64, 0:128],
                )
                nc.any.tensor_add(
                    acc_c[64:128, m : m + 1], acc_c[64:128, m : m + 1], tmp2[64:128, 128:129]
                )
                nc.any.tensor_add(
                    acc_c[0:64, m + 1 : m + 2], acc_c[0:64, m + 1 : m + 2], tmp2[0:64, 128:129]
                )

        nc.sync.dma_start(sums[:, :], acc[:])
        nc.sync.dma_start(cnts[:, :], acc_c[:])

    nc.compile()
    return nc


def _get_nc():
    if "nc" not in _CACHE:
        _CACHE["nc"] = _build_bass()
    return _CACHE["nc"]


def _get_runner():
    """Cached jitted 8-core runner (mirrors bass2jax.run_bass_via_pjrt)."""
    if "runner" in _CACHE:
        return _CACHE["runner"]

    import jax
    from jax.sharding import Mesh, PartitionSpec
    from jax.experimental.shard_map import shard_map
    from concourse import bass2jax, mybir

    nc = _get_nc()
    bass2jax.install_neuronx_cc_hook()

    partition_name = (
        nc.partition_id_tensor.name if nc.partition_id_tensor else None
    )
    in_names, out_names, out_avals, zero_outs = [], [], [], []
    for alloc in nc.m.functions[0].allocations:
        if not isinstance(alloc, mybir.MemoryLocationSet):
            continue
        name = alloc.memorylocations[0].name
        if alloc.kind == "ExternalInput":
            if name != partition_name:
                in_names.append(name)
        elif alloc.kind == "ExternalOutput":
            out_names.append(name)
            out_avals.append(
                jax.core.ShapedArray(alloc.tensor_shape, mybir.dt.np(alloc.dtype))
            )
            zero_outs.append(
                np.zeros(alloc.tensor_shape, dtype=mybir.dt.np(alloc.dtype))
            )

    n_params = len(in_names)
    n_outs = len(out_names)
    all_names = tuple(
        in_names + out_names + ([partition_name] if partition_name else [])
    )
    donate = tuple(range(n_params, n_params + n_outs))

    def _body(*args):
        operands = list(args)
        if partition_name:
            operands.append(bass2jax.partition_id_tensor())
        outs = bass2jax._bass_exec_p.bind(
            *operands,
            out_avals=tuple(out_avals),
            in_names=all_names,
            out_names=tuple(out_names),
            lowering_input_output_aliases=(),
            sim_require_finite=True,
            sim_require_nnan=True,
            nc=nc,
        )
        return tuple(outs)

    devices = jax.devices()[:NCORES]
    mesh = Mesh(np.asarray(devices), ("core",))
    sharded = jax.jit(
        shard_map(
            _body,
            mesh=mesh,
            in_specs=(PartitionSpec("core"),) * (n_params + n_outs),
            out_specs=(PartitionSpec("core"),) * n_outs,
            check_rep=False,
        ),
        donate_argnums=donate,
        keep_unused=True,
    )
    runner = (sharded, tuple(in_names), tuple(out_names), zero_outs)
    _CACHE["runner"] = runner
    return runner


def _run_device(concat_in):
    """concat_in: dict name -> (NCORES*128, ...) concatenated array (host or device).
    Returns dict name -> np.ndarray of shape (NCORES*128, ...) stacked outputs."""
    sharded, in_names, out_names, zero_outs = _get_runner()
    zeros = [
        np.zeros((NCORES * z.shape[0], *z.shape[1:]), z.dtype) for z in zero_outs
    ]
    out_arrs = sharded(*[concat_in[n] for n in in_names], *zeros)
    return {n: np.asarray(a) for n, a in zip(out_names, out_arrs)}


def _host_prep(feat, ids):
    """Returns (in_maps, ok). ok=False means window margins were violated."""
    # Window w covers global segs [64w - 32, 64w + 96); tile g belongs to
    # window g // 128. All ids of tile g must fall inside its window.
    g_base = 64 * (np.arange(N // 128, dtype=np.int64) // WT) - 32
    rel = ids.reshape(N // 128, 128) - g_base[:, None]
    if rel.min() < 0 or rel.max() > 127:
        return None, False
    # (ntiles, 128) -> concatenated per-core (NCORES*128, TPC)
    rel_cat = np.ascontiguousarray(
        rel.astype(np.float32).reshape(NCORES, TPC, 128).transpose(0, 2, 1)
    ).reshape(NCORES * 128, TPC)

    hi = feat.astype(BF16)
    lo = (feat - hi.astype(np.float32)).astype(BF16)
    hl = np.empty((N, ROW), dtype=BF16)
    hl[:, 0:128] = hi
    del hi
    hl[:, 128:256] = lo
    del lo
    hl[:, 256] = BF16(1.0)
    hl[:, 257] = BF16(0.0)
    # (N, ROW) -> per-core tiled (128, TPC*ROW): [p, t*ROW + c] = hl[128t + p, c]
    hl_cat = np.ascontiguousarray(
        hl.reshape(NCORES, TPC, 128, ROW).transpose(0, 2, 1, 3)
    ).reshape(NCORES * 128, TPC * ROW)
    del hl

    return {"hl": hl_cat, "rel": rel_cat}, True


def _numpy_fallback(feat, ids, num_segments):
    sums = np.zeros((num_segments, D), dtype=np.float32)
    np.add.at(sums, ids, feat)
    counts = np.bincount(ids, minlength=num_segments).astype(np.float32)
    return sums / np.maximum(counts, 1.0)[:, None]


def kernel(atom_features, segment_ids, num_segments):
    feat = np.asarray(atom_features, dtype=np.float32)
    ids = np.asarray(segment_ids, dtype=np.int64)
    nseg = int(num_segments)
    assert feat.shape == (N, D) and ids.shape == (N,) and nseg == B, (
        feat.shape,
        ids.shape,
        nseg,
    )

    concat_in, ok = _host_prep(feat, ids)
    if not ok:
        return _numpy_fallback(feat, ids, nseg)

    res = _run_device(concat_in)

    sums = np.zeros((B, D), dtype=np.float32)
    counts = np.zeros((B,), dtype=np.float32)
    for c in range(NCORES):
        # local seg ls = 128*b + p maps to global s = 1024c - 32 + ls
        loc = (
            res["sums"][128 * c : 128 * (c + 1)]
            .reshape(128, NBLK, 128)
            .transpose(1, 0, 2)
            .reshape(NBLK * 128, 128)
        )
        loc_c = (
            res["cnts"][128 * c : 128 * (c + 1)].transpose(1, 0).reshape(NBLK * 128)
        )
        s = 1024 * c - 32 + np.arange(NBLK * 128)
        valid = (s >= 0) & (s < B)
        sums[s[valid]] += loc[valid]
        counts[s[valid]] += loc_c[valid]

    return sums / np.maximum(counts, 1.0)[:, None]
